# revision 1
# baseline (speedup 1.0000x reference)
"""Anisotropic collisions kernel for 8 TRN2 NeuronCores.

Math: for each of 9*64*64 = 36864 independent systems (mode, spatial cell),
build tridiagonal coefficients from Rosenbluth cumulative integrals of
flm(v) along v (512 points), then solve the tridiagonal system along v.

Reformulation (validated numerically, rel err ~5e-6 vs f64 Thomas):
  G1 = cumsum(y*g1(v)) + 2*S1,  g1 = 3v^2 - v^4 - 2v
  G2 = cumsum(y*g2(v)) + S1,    g2 = v^4 - v
  S1 = sum(y*v)
  w = G1*KY/(2*DV*v^3); u = G2*KY/(DV^2*v^2); KY = 4*pi*Y_DT/3
  a = u - w; c = u + w; b = 1 + 8*pi*Y_DT*y + u/2 - il2*(2*DV/v)*w
Solve via scan-form Thomas: cp ~= c/(b - a*shift(c/b)) (one fixed-point
refinement of the continued fraction -- strongly diagonally dominant since
Y_DT=1e-12), then dp and back-substitution are first-order linear
recurrences computed with tensor_tensor_scan.

Layout: batch on partitions, v along free. Each SBUF tile [128, 4*512]
holds 512 systems (4 consecutive shard rows per partition). Scan "resets"
at system boundaries come from zeros in the scan multiplier column (a[v=0]
and cp[v=511] are unused by Thomas, so zeroing them is exact).

Toolchain notes: this walrus build accepts only ONE sync-wait per
instruction and rejects custom-DVE InstISA ops, so we use standard ISA ops
only and split multi-wait instructions into standalone InstEventSemaphore
waits in a post-pass.
"""

import numpy as np
from contextlib import ExitStack

import concourse.bass as bass
import concourse.tile as tile
import concourse.mybir as mybir
from concourse.bass_utils import run_bass_kernel_spmd

F32 = mybir.dt.float32
BF16 = mybir.dt.bfloat16

NX, NY, NV = 64, 64, 512
N_MODES = 9
DV = 0.015625
Y_DT = 1.0e-12
FOUR_PI = 4.0 * np.pi
KY = FOUR_PI * Y_DT / 3.0

N_CORES = 8
ROWS_TOTAL = N_MODES * NX * NY            # 36864
ROWS_PER_CORE = ROWS_TOTAL // N_CORES     # 4608
FUSE = 4                                  # systems per partition row
GROUP_ROWS = 128 * FUSE                   # 512 systems per group
N_GROUPS = ROWS_PER_CORE // GROUP_ROWS    # 9
FD = FUSE * NV                            # 2048

REFINE = False                            # one cp fixed-point refinement
BF16_TAIL = False                         # bf16 solve tail (no model-predicted gain; f32 keeps 2.9e-3 accuracy)

_V = (np.arange(NV, dtype=np.float64) + 1.0) * DV

# constant profile blob layout (each [128, FD] f32, rows replicated):
_C_NAMES = ["resetv", "reset1", "g1w", "g2w", "t1c", "pw2k", "pu2k"]  # reset1=r3, pw2k=r2 (ratio vectors)
NC_CONST = len(_C_NAMES)


def _profiles():
    v = _V
    g1w = 3.0 * v**2 - v**4 - 2.0 * v
    g2w = v**4 - v
    t1c = -2.0 * DV / v                   # t1 = wn*t1c = +coeff1*Y_DT/v
    pwn = -KY / (2.0 * DV * v**3)         # wn = (G1b + 2 S1)*pwn = -w
    pun = -KY / (DV * DV * v**2)          # un = (G2b + S1)*pun = -u
    r1 = np.ones(NV)
    r1[1:] = v[:-1] / v[1:]
    r1[0] = 0.0                           # scan reset at each system start
    ones0 = np.ones(NV)
    ones0[0] = 0.0
    r3 = np.ones(NV)
    r3[1:] = (v[:-1] / v[1:])**3          # pw_t/pw_{t-1}
    r3[0] = 0.0
    r2 = np.ones(NV)
    r2[1:] = (v[:-1] / v[1:])**2          # pu_t/pu_{t-1}
    r2[0] = 0.0
    prof = {
        "g1w": g1w * pwn, "g2w": g2w * pun, "t1c": t1c,
        "resetv": r1, "reset1": r3, "pw2k": r2, "pu2k": pun,
    }
    return np.concatenate([np.tile(prof[n], FUSE) for n in _C_NAMES])


def _legalize_multiwait(nc):
    """Split instructions with >1 sync wait: keep one wait on the
    instruction, hoist the rest onto standalone InstEventSemaphore ops
    immediately before it on the same engine (this walrus accepts only one
    wait per instruction)."""
    n = [0]

    def fresh(engine, wait):
        n[0] += 1
        return mybir.InstEventSemaphore(
            name=f"mwsplit-{n[0]}",
            engine=engine,
            sync_info=mybir.SyncInfo(on_wait=[wait], on_update=[]),
        )

    for fn in nc.m.functions:
        for blk in fn.blocks:
            out = []
            for ins in blk.instructions:
                si = ins.sync_info
                if si is not None and si.on_wait is not None and len(si.on_wait) > 1:
                    waits = list(si.on_wait)
                    for w in waits[:-1]:
                        out.append(fresh(ins.engine, w))
                    si.on_wait = [waits[-1]]
                out.append(ins)
            blk.instructions[:] = out


def build_nc(n_groups=N_GROUPS, legalize=True, repeat=1):
    nc = bass.Bass()
    rows = n_groups * GROUP_ROWS
    y_in = nc.declare_dram_parameter("y", [rows, NV], F32, isOutput=False)
    il2_in = nc.declare_dram_parameter("il2", [128, n_groups], F32, isOutput=False)
    cst_in = nc.declare_dram_parameter("cst", [128, NC_CONST * FD], F32, isOutput=False)
    out_ext = nc.declare_dram_parameter("out", [rows, NV], F32, isOutput=True)

    MUL = mybir.AluOpType.mult
    ADD = mybir.AluOpType.add
    SUB = mybir.AluOpType.subtract
    COPY = mybir.ActivationFunctionType.Copy

    with ExitStack() as ctx:
        tc = ctx.enter_context(tile.TileContext(nc))
        cpool = ctx.enter_context(tc.tile_pool(name="consts", bufs=1))

        cst = cpool.tile([128, NC_CONST * FD], F32, tag="cst")
        # three concurrent const segments: scan consts land first so the
        # first group's scans start ~6us earlier
        segs = [(0, 2 * FD), (2 * FD, 4 * FD), (4 * FD, NC_CONST * FD)]
        for i, (lo, hi) in enumerate(segs):
            nc.gpsimd.dma_start(cst[:, lo:hi], cst_in[:, lo:hi])
        C = {nm: cst[:, i * FD:(i + 1) * FD] for i, nm in enumerate(_C_NAMES)}
        for i, (lo, hi) in enumerate(segs):
            tch = cpool.tile([128, 1], F32, tag=f"touch_{i}")
            nc.vector.tensor_copy(out=tch[:, :], in_=cst[:, lo:lo + 1])
        io = ctx.enter_context(tc.tile_pool(name="io", bufs=2))
        wk = ctx.enter_context(tc.tile_pool(name="work", bufs=1))
        il2t = cpool.tile([128, n_groups], F32, tag="il2")
        nc.gpsimd.dma_start(il2t[:, :], il2_in[:, :])
        touch_b = cpool.tile([128, 1], F32, tag="touch_b")
        nc.vector.tensor_copy(out=touch_b[:, :], in_=il2t[:, 0:1])

        for rep in range(repeat):
          for g in range(n_groups):
            rsl = slice(g * GROUP_ROWS, (g + 1) * GROUP_ROWS)
            y_src = y_in[rsl, :].rearrange("(p j) v -> p (j v)", p=128)
            x_dst = out_ext[rsl, :].rearrange("(p j) v -> p (j v)", p=128)

            y4 = io.tile([128, FD], F32, tag="y4")
            nc.gpsimd.dma_start(y4[:, :], y_src)

            # t3 = 1 + 8*pi*Y_DT*y   (ACT)
            t3 = io.tile([128, FD], F32, tag="t3")
            nc.scalar.activation(t3[:, :], y4[:, :], COPY,
                                 bias=1.0, scale=float(8.0 * np.pi * Y_DT))

            wg1 = wk.tile([128, FD], F32, tag="T1")
            nc.vector.tensor_tensor(out=wg1[:, :], in0=y4[:, :], in1=C["g1w"], op=MUL)
            wg2 = wk.tile([128, FD], F32, tag="T2")
            nc.vector.tensor_tensor(out=wg2[:, :], in0=y4[:, :], in1=C["g2w"], op=MUL)

            # E1 = C1/v per system (ratio scan); S1 = E1[v_last] * v_last
            E1 = wk.tile([128, FD], F32, tag="T3")
            nc.vector.tensor_tensor_scan(E1[:, :], C["resetv"], y4[:, :], 0.0,
                                         op0=MUL, op1=ADD)
            s1x = wk.tile([128, FUSE], F32, tag="s1x")
            nc.scalar.activation(s1x[:, :], E1[:, NV - 1::NV], COPY,
                                 bias=0.0, scale=float(_V[-1]))
            pw0 = float(-KY / (2.0 * DV * _V[0]**3))
            pu0 = float(-KY / (DV * DV * _V[0]**2))
            s1x2 = wk.tile([128, FUSE], F32, tag="s1x2")
            nc.scalar.activation(s1x2[:, :], s1x[:, :], COPY, bias=0.0,
                                 scale=2.0 * pw0)
            s1xp = wk.tile([128, FUSE], F32, tag="s1xp")
            nc.scalar.activation(s1xp[:, :], s1x[:, :], COPY, bias=0.0, scale=pu0)

            # Inject the (weight-folded) S1 terms at each system's first
            # column: the weighted ratio-scans then carry pw*(G1b+2*S1) and
            # pu*(G2b+S1) directly.
            nc.vector.tensor_tensor(out=wg1[:, 0::NV], in0=wg1[:, 0::NV],
                                    in1=s1x2[:, :], op=ADD)
            nc.vector.tensor_tensor(out=wg2[:, 0::NV], in0=wg2[:, 0::NV],
                                    in1=s1xp[:, :], op=ADD)
            wn = wk.tile([128, FD], F32, tag="T4")    # = -w (ratio scan)
            nc.vector.tensor_tensor_scan(wn[:, :], C["reset1"], wg1[:, :], 0.0,
                                         op0=MUL, op1=ADD)
            un = wk.tile([128, FD], F32, tag="T5")    # = -u (ratio scan)
            nc.vector.tensor_tensor_scan(un[:, :], C["pw2k"], wg2[:, :], 0.0,
                                         op0=MUL, op1=ADD)

            TD0 = BF16 if BF16_TAIL else F32
            a_pos = wk.tile([128, FD], TD0, tag="T1b")   # a = u - w
            nc.vector.tensor_tensor(out=a_pos[:, :], in0=wn[:, :], in1=un[:, :], op=SUB)
            c_pos = wk.tile([128, FD], TD0, tag="T2b")   # c = u + w
            nc.vector.scalar_tensor_tensor(out=c_pos[:, :], in0=un[:, :], scalar=-1.0,
                                           in1=wn[:, :], op0=MUL, op1=SUB)
            t1 = wk.tile([128, FD], F32, tag="T1")      # +coeff1*Y/v
            nc.vector.tensor_tensor(out=t1[:, :], in0=wn[:, :], in1=C["t1c"], op=MUL)
            b1 = wk.tile([128, FD], F32, tag="T2")      # t3 + u/2
            nc.vector.scalar_tensor_tensor(out=b1[:, :], in0=un[:, :], scalar=-0.5,
                                           in1=t3[:, :], op0=MUL, op1=ADD)
            bn = wk.tile([128, FD], F32, tag="T5")      # -b
            nc.vector.scalar_tensor_tensor(out=bn[:, :], in0=t1[:, :],
                                           scalar=il2t[:, g:g + 1],
                                           in1=b1[:, :], op0=MUL, op1=SUB)
            binv_n = wk.tile([128, FD], F32, tag="T8")  # -1/b
            nc.vector.reciprocal(out=binv_n[:, :], in_=bn[:, :])

            if REFINE:
                # den = b - a*shift(c/b); dinv_n = -1/den
                mcp0g = wk.tile([128, FD + 1], F32, tag="T9")
                nc.vector.memset(mcp0g[:, 0:1], 0.0)
                nc.vector.tensor_tensor(out=mcp0g[:, 1:FD + 1], in0=c_pos[:, :],
                                        in1=binv_n[:, :], op=MUL)  # -cp0
                tpp = wk.tile([128, FD], F32, tag="T1")
                nc.vector.tensor_tensor(out=tpp[:, :], in0=a_pos[:, :],
                                        in1=mcp0g[:, 0:FD], op=MUL)  # -a*cp0sh
                tppv = tpp[:, :].rearrange("p (j v) -> p j v", j=FUSE)
                nc.vector.memset(tppv[:, :, 0:1], 0.0)
                den_n = wk.tile([128, FD], F32, tag="T2")
                nc.vector.tensor_tensor(out=den_n[:, :], in0=bn[:, :],
                                        in1=tpp[:, :], op=SUB)  # -den
                dinv_n = wk.tile([128, FD], F32, tag="T8")
                nc.vector.reciprocal(out=dinv_n[:, :], in_=den_n[:, :])  # -1/den
            else:
                dinv_n = binv_n

            TD = BF16 if BF16_TAIL else F32
            if BF16_TAIL:
                dinv_b = wk.tile([128, FD], BF16, tag="T9b")
                nc.vector.tensor_copy(out=dinv_b[:, :], in_=dinv_n[:, :])
                y_b = wk.tile([128, FD], BF16, tag="T10b")
                nc.vector.tensor_copy(out=y_b[:, :], in_=y4[:, :])
            else:
                dinv_b, y_b = dinv_n, y4
            alpha = wk.tile([128, FD], TD, tag="T1")    # -a/den
            nc.vector.tensor_tensor(out=alpha[:, :], in0=a_pos[:, :],
                                    in1=dinv_b[:, :], op=MUL)
            av = alpha[:, :].rearrange("p (j v) -> p j v", j=FUSE)
            nc.vector.memset(av[:, :, 0:1], 0.0)        # scan reset at v=0
            beta = wk.tile([128, FD], TD, tag="T5")     # +d/den
            nc.vector.scalar_tensor_tensor(out=beta[:, :], in0=dinv_b[:, :],
                                           scalar=-1.0, in1=y_b[:, :],
                                           op0=MUL, op1=MUL)
            mcp = wk.tile([128, FD], TD, tag="T2")      # -c/den
            nc.vector.tensor_tensor(out=mcp[:, :], in0=c_pos[:, :],
                                    in1=dinv_b[:, :], op=MUL)
            mv = mcp[:, :].rearrange("p (j v) -> p j v", j=FUSE)
            nc.vector.memset(mv[:, :, NV - 1:NV], 0.0)  # bwd scan reset at v=511

            dp = wk.tile([128, FD], TD, tag="T10")
            nc.vector.tensor_tensor_scan(dp[:, :], alpha[:, :], beta[:, :], 0.0,
                                         op0=MUL, op1=ADD)
            x4 = io.tile([128, FD], F32, tag="x4")
            nc.vector.tensor_tensor_scan(x4[:, ::-1], mcp[:, ::-1], dp[:, ::-1], 0.0,
                                         op0=MUL, op1=ADD)
            nc.gpsimd.dma_start(x_dst, x4[:, :])

    if legalize:
        _legalize_multiwait(nc)
    return nc


_NC_CACHE = {}


def _get_nc(n_groups=N_GROUPS):
    if n_groups not in _NC_CACHE:
        _NC_CACHE[n_groups] = build_nc(n_groups)
    return _NC_CACHE[n_groups]


_CST_CACHE = None


def make_inputs(y_shard, il2_rows, n_groups=N_GROUPS):
    """Per-core input map. y_shard [rows, 512] f32; il2_rows [rows] f32."""
    global _CST_CACHE
    if _CST_CACHE is None:
        _CST_CACHE = np.broadcast_to(_profiles()[None, :], (128, NC_CONST * FD)
                                     ).astype(np.float32).copy()
    cst = _CST_CACHE
    il2 = il2_rows.reshape(n_groups, 128, FUSE)[:, :, 0].T.astype(np.float32).copy()
    return {
        "y": np.ascontiguousarray(y_shard, dtype=np.float32),
        "il2": il2,
        "cst": cst,
    }


def kernel(y, il_arr):
    y = np.asarray(y, dtype=np.float32)
    il_arr = np.asarray(il_arr)
    yf = y.reshape(ROWS_TOTAL, NV)
    il_f = il_arr.astype(np.float64)
    il2_all = np.repeat(il_f * (il_f + 1.0) / 2.0, NX * NY).astype(np.float32)

    nc = _get_nc()
    in_maps = []
    for c in range(N_CORES):
        rs = slice(c * ROWS_PER_CORE, (c + 1) * ROWS_PER_CORE)
        in_maps.append(make_inputs(yf[rs], il2_all[rs]))
    res = run_bass_kernel_spmd(nc, in_maps, core_ids=list(range(N_CORES)))
    outs = [res.results[c]["out"] for c in range(N_CORES)]
    x = np.concatenate(outs, axis=0).reshape(N_MODES, NX, NY, NV)
    return x.astype(np.float32)



# revision 3
# speedup vs baseline: 4.7643x; 4.7643x over previous
"""Anisotropic collisions kernel for 8 TRN2 NeuronCores.

Math: for each of 9*64*64 = 36864 independent systems (mode, spatial cell),
build tridiagonal coefficients from Rosenbluth cumulative integrals of
flm(v) along v (512 points), then solve the tridiagonal system along v.

Key structural facts exploited (validated numerically vs f64 Thomas):
  1. The collision coefficients u (c2-term) and w (c1-term) decay ~1/v^2;
     beyond v-index T0 the tridiagonal system is identity to ~1e-4 * x.
     The solve therefore runs only on the first T0 columns of each
     512-system ("head"); the tail passes through (x = y) via an in-place
     scatter of the head solution into the input tile followed by one
     contiguous output DMA. Only S1 = sum(y*v) needs the full row: one
     full-length ratio scan (E1) on DVE.
  2. Thomas without the cp refinement (cp = c/b) is accurate to ~3e-3.

Scheduling: input DMA rides the SP queue, output DMA the Pool queue
(transfers on different queues overlap in time). Scans + reciprocal are
DVE-only ops; every elementwise tensor_tensor runs on the Pool engine
(flat-rate ALU, otherwise idle); activations (scaled copies) run on ACT.
Scale factors are folded into host-precomputed profiles so no
tensor_scalar / scalar_tensor_tensor is needed (TensorScalarPtr is
DVE-only on this toolchain): the weighted scans emit -w/2 and -u/2
directly, and the il2*(2DV/v) diagonal term uses a per-group outer
product profile il2[p] * 4DV/v[f].

Toolchain notes: this walrus build accepts only ONE sync-wait per
instruction; multi-wait instructions are split into standalone
InstEventSemaphore waits in a post-pass.
"""

import numpy as np
from contextlib import ExitStack

import concourse.bass as bass
import concourse.tile as tile
import concourse.mybir as mybir
from concourse.bass_utils import run_bass_kernel_spmd

F32 = mybir.dt.float32

NX, NY, NV = 64, 64, 512
N_MODES = 9
DV = 0.015625
Y_DT = 1.0e-12
FOUR_PI = 4.0 * np.pi
KY = FOUR_PI * Y_DT / 3.0

N_CORES = 8
ROWS_TOTAL = N_MODES * NX * NY            # 36864
ROWS_PER_CORE = ROWS_TOTAL // N_CORES     # 4608
FUSE = 4                                  # systems per partition row
GROUP_ROWS = 128 * FUSE                   # 512 systems per group
N_GROUPS = ROWS_PER_CORE // GROUP_ROWS    # 9
FD = FUSE * NV                            # 2048
T0 = 64                                   # head length per system
HD = FUSE * T0

_V = (np.arange(NV, dtype=np.float64) + 1.0) * DV

# f32 const blob: resetv [FD], then reset1h, pw2kh, g1wh, g2wh [HD each]
CF_W = FD + 4 * HD


def _profiles():
    v = _V
    vh = v[:T0]
    g1 = 3.0 * v**2 - v**4 - 2.0 * v
    g2 = v**4 - v
    pwn = -KY / (2.0 * DV * v**3)         # wn' = -w/2  (0.5 folded in)
    pun = -KY / (DV * DV * v**2)          # un' = -u/2
    r1 = np.ones(NV)
    r1[1:] = v[:-1] / v[1:]
    r1[0] = 0.0                           # E1 reset at each system start
    r3 = np.ones(T0)
    r3[1:] = (vh[:-1] / vh[1:])**3
    r3[0] = 0.0
    r2 = np.ones(T0)
    r2[1:] = (vh[:-1] / vh[1:])**2
    r2[0] = 0.0
    return np.concatenate([
        np.tile(r1, FUSE),
        np.tile(r3, FUSE),
        np.tile(r2, FUSE),
        np.tile(0.5 * g1[:T0] * pwn[:T0], FUSE),
        np.tile(0.5 * g2[:T0] * pun[:T0], FUSE),
    ])


def _legalize_multiwait(nc):
    """Split instructions with >1 sync wait: keep one wait on the
    instruction, hoist the rest onto standalone InstEventSemaphore ops
    immediately before it on the same engine (this walrus accepts only one
    wait per instruction)."""
    n = [0]

    def fresh(engine, wait):
        n[0] += 1
        return mybir.InstEventSemaphore(
            name=f"mwsplit-{n[0]}",
            engine=engine,
            sync_info=mybir.SyncInfo(on_wait=[wait], on_update=[]),
        )

    for fn in nc.m.functions:
        for blk in fn.blocks:
            out = []
            for ins in blk.instructions:
                si = ins.sync_info
                if si is not None and si.on_wait is not None and len(si.on_wait) > 1:
                    waits = list(si.on_wait)
                    for w in waits[:-1]:
                        out.append(fresh(ins.engine, w))
                    si.on_wait = [waits[-1]]
                out.append(ins)
            blk.instructions[:] = out


def build_nc(n_groups=N_GROUPS, legalize=True):
    nc = bass.Bass()
    rows = n_groups * GROUP_ROWS
    y_in = nc.declare_dram_parameter("y", [rows, NV], F32, isOutput=False)
    cf_in = nc.declare_dram_parameter("cf", [128, CF_W], F32, isOutput=False)
    ilp_in = nc.declare_dram_parameter("ilp", [128, n_groups * HD], F32, isOutput=False)
    out_ext = nc.declare_dram_parameter("out", [rows, NV], F32, isOutput=True)

    MUL = mybir.AluOpType.mult
    ADD = mybir.AluOpType.add
    SUB = mybir.AluOpType.subtract
    COPY = mybir.ActivationFunctionType.Copy

    pw0 = float(-KY / (2.0 * DV * _V[0]**3))
    pu0 = float(-KY / (DV * DV * _V[0]**2))
    vlast = float(_V[-1])

    with ExitStack() as ctx:
        tc = ctx.enter_context(tile.TileContext(nc))
        cpool = ctx.enter_context(tc.tile_pool(name="consts", bufs=1))

        cf = cpool.tile([128, CF_W], F32, tag="cf")
        nc.scalar.dma_start(cf[:, :], cf_in[:, :])
        ilp = cpool.tile([128, n_groups * HD], F32, tag="ilp")
        nc.scalar.dma_start(ilp[:, :], ilp_in[:, :])

        resetv = cf[:, 0:FD]
        reset1h = cf[:, FD:FD + HD]
        pw2kh = cf[:, FD + HD:FD + 2 * HD]
        g1wh = cf[:, FD + 2 * HD:FD + 3 * HD]
        g2wh = cf[:, FD + 3 * HD:FD + 4 * HD]

        # touch consts so the tile framework orders compute after the loads
        for nm, seg in (("tc_f", cf), ("tc_i", ilp)):
            tch = cpool.tile([128, 1], F32, tag=nm)
            nc.vector.tensor_copy(out=tch[:, :], in_=seg[:, 0:1])

        io = ctx.enter_context(tc.tile_pool(name="io", bufs=3))
        e1p = ctx.enter_context(tc.tile_pool(name="e1", bufs=2))
        wk = ctx.enter_context(tc.tile_pool(name="work", bufs=2))

        for g in range(n_groups):
            rsl = slice(g * GROUP_ROWS, (g + 1) * GROUP_ROWS)
            y_src = y_in[rsl, :].rearrange("(p j) v -> p (j v)", p=128)
            x_dst = out_ext[rsl, :].rearrange("(p j) v -> p (j v)", p=128)

            y4 = io.tile([128, FD], F32, tag="y4")
            nc.sync.dma_start(y4[:, :], y_src)
            y4v = y4[:, :].rearrange("p (j v) -> p j v", j=FUSE)

            # S1 per system via full-row ratio scan (E1_t = P(y*v)_t / v_t)
            E1 = e1p.tile([128, FD], F32, tag="E1")
            nc.vector.tensor_tensor_scan(E1[:, :], resetv, y4[:, :], 0.0,
                                         op0=MUL, op1=ADD)
            # S1 seeds, scaled for the half-unit weighted scans
            s1x2 = wk.tile([128, FUSE], F32, tag="s1x2")
            nc.scalar.activation(s1x2[:, :], E1[:, NV - 1::NV], COPY,
                                 bias=0.0, scale=pw0 * vlast)
            s1xp = wk.tile([128, FUSE], F32, tag="s1xp")
            nc.scalar.activation(s1xp[:, :], E1[:, NV - 1::NV], COPY,
                                 bias=0.0, scale=0.5 * pu0 * vlast)

            # head compaction
            yh = wk.tile([128, HD], F32, tag="yh")
            yhv = yh[:, :].rearrange("p (j v) -> p j v", j=FUSE)
            nc.gpsimd.tensor_copy(out=yhv[:, :, :], in_=y4v[:, :, 0:T0])

            wg1 = wk.tile([128, HD], F32, tag="wg1")
            nc.gpsimd.tensor_tensor(out=wg1[:, :], in0=yh[:, :], in1=g1wh, op=MUL)
            wg2 = wk.tile([128, HD], F32, tag="wg2")
            nc.gpsimd.tensor_tensor(out=wg2[:, :], in0=yh[:, :], in1=g2wh, op=MUL)
            # inject S1 seeds at each system's first head column
            nc.gpsimd.tensor_tensor(out=wg1[:, 0::T0], in0=wg1[:, 0::T0],
                                    in1=s1x2[:, :], op=ADD)
            nc.gpsimd.tensor_tensor(out=wg2[:, 0::T0], in0=wg2[:, 0::T0],
                                    in1=s1xp[:, :], op=ADD)

            # weighted ratio scans: wn = -w/2, un = -u/2
            wn = wk.tile([128, HD], F32, tag="wn")
            nc.vector.tensor_tensor_scan(wn[:, :], reset1h, wg1[:, :], 0.0,
                                         op0=MUL, op1=ADD)
            un = wk.tile([128, HD], F32, tag="un")
            nc.vector.tensor_tensor_scan(un[:, :], pw2kh, wg2[:, :], 0.0,
                                         op0=MUL, op1=ADD)

            # diagonal b = t3 + u/2 - il2*(2DV/v)*w
            t3 = wk.tile([128, HD], F32, tag="t3")
            nc.scalar.activation(t3[:, :], yh[:, :], COPY,
                                 bias=1.0, scale=float(8.0 * np.pi * Y_DT))
            b1 = wk.tile([128, HD], F32, tag="b1")
            nc.gpsimd.tensor_tensor(out=b1[:, :], in0=t3[:, :], in1=un[:, :], op=SUB)
            bil = wk.tile([128, HD], F32, tag="bil")
            nc.gpsimd.tensor_tensor(out=bil[:, :], in0=wn[:, :],
                                    in1=ilp[:, g * HD:(g + 1) * HD], op=MUL)
            bpos = wk.tile([128, HD], F32, tag="bpos")
            nc.gpsimd.tensor_tensor(out=bpos[:, :], in0=b1[:, :], in1=bil[:, :], op=ADD)
            binv = wk.tile([128, HD], F32, tag="binv")
            nc.vector.reciprocal(out=binv[:, :], in_=bpos[:, :])
            binv2 = wk.tile([128, HD], F32, tag="binv2")
            nc.scalar.activation(binv2[:, :], binv[:, :], COPY, bias=0.0, scale=2.0)

            # off-diagonals: a_n = -a/2, c_n = -c/2
            a_n = wk.tile([128, HD], F32, tag="a_n")
            nc.gpsimd.tensor_tensor(out=a_n[:, :], in0=un[:, :], in1=wn[:, :], op=SUB)
            c_n = wk.tile([128, HD], F32, tag="c_n")
            nc.gpsimd.tensor_tensor(out=c_n[:, :], in0=un[:, :], in1=wn[:, :], op=ADD)

            alpha = wk.tile([128, HD], F32, tag="alpha")    # -a/b
            nc.gpsimd.tensor_tensor(out=alpha[:, :], in0=a_n[:, :], in1=binv2[:, :], op=MUL)
            av = alpha[:, :].rearrange("p (j v) -> p j v", j=FUSE)
            nc.vector.memset(av[:, :, 0:1], 0.0)            # fwd scan reset
            mcp = wk.tile([128, HD], F32, tag="mcp")        # -c/b
            nc.gpsimd.tensor_tensor(out=mcp[:, :], in0=c_n[:, :], in1=binv2[:, :], op=MUL)
            mv = mcp[:, :].rearrange("p (j v) -> p j v", j=FUSE)
            nc.vector.memset(mv[:, :, T0 - 1:T0], 0.0)      # bwd scan reset
            beta = wk.tile([128, HD], F32, tag="beta")      # y/b
            nc.gpsimd.tensor_tensor(out=beta[:, :], in0=yh[:, :], in1=binv[:, :], op=MUL)

            dp = wk.tile([128, HD], F32, tag="dp")
            nc.vector.tensor_tensor_scan(dp[:, :], alpha[:, :], beta[:, :], 0.0,
                                         op0=MUL, op1=ADD)
            xh = wk.tile([128, HD], F32, tag="xh")
            nc.vector.tensor_tensor_scan(xh[:, ::-1], mcp[:, ::-1], dp[:, ::-1], 0.0,
                                         op0=MUL, op1=ADD)

            # scatter head solution into the input tile; tail passes through
            xhv = xh[:, :].rearrange("p (j v) -> p j v", j=FUSE)
            nc.gpsimd.tensor_copy(out=y4v[:, :, 0:T0], in_=xhv[:, :, :])
            nc.gpsimd.dma_start(x_dst, y4[:, :])

    if legalize:
        _legalize_multiwait(nc)
    return nc


_NC_CACHE = {}


def _get_nc(n_groups=N_GROUPS):
    if n_groups not in _NC_CACHE:
        _NC_CACHE[n_groups] = build_nc(n_groups)
    return _NC_CACHE[n_groups]


_CF_CACHE = None


def make_inputs(y_shard, il2_rows, n_groups=N_GROUPS):
    """Per-core input map. y_shard [rows, 512] f32; il2_rows [rows] f32
    (holding il*(il+1)/2 per row)."""
    global _CF_CACHE
    if _CF_CACHE is None:
        _CF_CACHE = np.broadcast_to(_profiles()[None, :], (128, CF_W)
                                    ).astype(np.float32).copy()
    il2 = il2_rows.reshape(n_groups, 128, FUSE)[:, :, 0]          # [g, 128]
    prof = (4.0 * DV / _V[:T0]).astype(np.float64)                # [T0]
    ilp = (il2[:, :, None, None] * prof[None, None, None, :])     # [g,128,1,T0]
    ilp = np.broadcast_to(ilp, (n_groups, 128, FUSE, T0))
    ilp = ilp.transpose(1, 0, 2, 3).reshape(128, n_groups * HD).astype(np.float32)
    return {
        "y": np.ascontiguousarray(y_shard, dtype=np.float32),
        "cf": _CF_CACHE,
        "ilp": np.ascontiguousarray(ilp),
    }


def kernel(y, il_arr):
    y = np.asarray(y, dtype=np.float32)
    il_arr = np.asarray(il_arr)
    yf = y.reshape(ROWS_TOTAL, NV)
    il_f = il_arr.astype(np.float64)
    il2_all = np.repeat(il_f * (il_f + 1.0) / 2.0, NX * NY).astype(np.float32)

    nc = _get_nc()
    in_maps = []
    for c in range(N_CORES):
        rs = slice(c * ROWS_PER_CORE, (c + 1) * ROWS_PER_CORE)
        in_maps.append(make_inputs(yf[rs], il2_all[rs]))
    res = run_bass_kernel_spmd(nc, in_maps, core_ids=list(range(N_CORES)))
    outs = [res.results[c]["out"] for c in range(N_CORES)]
    x = np.concatenate(outs, axis=0).reshape(N_MODES, NX, NY, NV)
    return x.astype(np.float32)


# revision 4
# speedup vs baseline: 5.6014x; 1.1757x over previous
"""Anisotropic collisions kernel for 8 TRN2 NeuronCores.

Math: for each of 9*64*64 = 36864 independent systems (mode, spatial cell),
build tridiagonal coefficients from Rosenbluth cumulative integrals of
flm(v) along v (512 points), then solve the tridiagonal system along v.

Key structural facts exploited (validated numerically vs f64 Thomas):
  1. The collision coefficients u (c2-term) and w (c1-term) decay ~1/v^2;
     beyond v-index T0 the tridiagonal system is identity to ~1e-4 * x.
     The solve therefore runs only on the first T0 columns of each
     512-system ("head"); the tail passes through (x = y) via an in-place
     scatter of the head solution into the input tile followed by one
     contiguous output DMA. Only S1 = sum(y*v) needs the full row: one
     full-length ratio scan (E1) on DVE.
  2. Thomas without the cp refinement (cp = c/b) is accurate to ~3e-3.

Scheduling: input DMA rides the SP queue, output DMA the Pool queue
(transfers on different queues overlap in time). Scans + reciprocal are
DVE-only ops; every elementwise tensor_tensor runs on the Pool engine
(flat-rate ALU, otherwise idle); activations (scaled copies) run on ACT.
Scale factors are folded into host-precomputed profiles so no
tensor_scalar / scalar_tensor_tensor is needed (TensorScalarPtr is
DVE-only on this toolchain): the weighted scans emit -w/2 and -u/2
directly, and the il2*(2DV/v) diagonal term uses a per-group outer
product profile il2[p] * 4DV/v[f].

Toolchain notes: this walrus build accepts only ONE sync-wait per
instruction; multi-wait instructions are split into standalone
InstEventSemaphore waits in a post-pass.
"""

import numpy as np
from contextlib import ExitStack

import concourse.bass as bass
import concourse.tile as tile
import concourse.mybir as mybir
from concourse.bass_utils import run_bass_kernel_spmd

F32 = mybir.dt.float32

NX, NY, NV = 64, 64, 512
N_MODES = 9
DV = 0.015625
Y_DT = 1.0e-12
FOUR_PI = 4.0 * np.pi
KY = FOUR_PI * Y_DT / 3.0

N_CORES = 8
ROWS_TOTAL = N_MODES * NX * NY            # 36864
ROWS_PER_CORE = ROWS_TOTAL // N_CORES     # 4608
FUSE = 4                                  # systems per partition row
GROUP_ROWS = 128 * FUSE                   # 512 systems per group
N_GROUPS = ROWS_PER_CORE // GROUP_ROWS    # 9
FD = FUSE * NV                            # 2048
T0 = 32                                   # head length per system
HD = FUSE * T0

_V = (np.arange(NV, dtype=np.float64) + 1.0) * DV

# f32 const blob: resetv [FD], then reset1h, pw2kh, g1wh, g2wh [HD each]
CF_W = FD + 4 * HD


def _profiles():
    v = _V
    vh = v[:T0]
    g1 = 3.0 * v**2 - v**4 - 2.0 * v
    g2 = v**4 - v
    pwn = -KY / (2.0 * DV * v**3)         # wn' = -w/2  (0.5 folded in)
    pun = -KY / (DV * DV * v**2)          # un' = -u/2
    r1 = np.ones(NV)
    r1[1:] = v[:-1] / v[1:]
    r1[0] = 0.0                           # E1 reset at each system start
    r3 = np.ones(T0)
    r3[1:] = (vh[:-1] / vh[1:])**3
    r3[0] = 0.0
    r2 = np.ones(T0)
    r2[1:] = (vh[:-1] / vh[1:])**2
    r2[0] = 0.0
    return np.concatenate([
        np.tile(r1, FUSE),
        np.tile(r3, FUSE),
        np.tile(r2, FUSE),
        np.tile(0.5 * g1[:T0] * pwn[:T0], FUSE),
        np.tile(0.5 * g2[:T0] * pun[:T0], FUSE),
    ])


def _legalize_multiwait(nc):
    """Split instructions with >1 sync wait: keep one wait on the
    instruction, hoist the rest onto standalone InstEventSemaphore ops
    immediately before it on the same engine (this walrus accepts only one
    wait per instruction)."""
    n = [0]

    def fresh(engine, wait):
        n[0] += 1
        return mybir.InstEventSemaphore(
            name=f"mwsplit-{n[0]}",
            engine=engine,
            sync_info=mybir.SyncInfo(on_wait=[wait], on_update=[]),
        )

    for fn in nc.m.functions:
        for blk in fn.blocks:
            out = []
            for ins in blk.instructions:
                si = ins.sync_info
                if si is not None and si.on_wait is not None and len(si.on_wait) > 1:
                    waits = list(si.on_wait)
                    for w in waits[:-1]:
                        out.append(fresh(ins.engine, w))
                    si.on_wait = [waits[-1]]
                out.append(ins)
            blk.instructions[:] = out


def build_nc(n_groups=N_GROUPS, legalize=True):
    nc = bass.Bass()
    rows = n_groups * GROUP_ROWS
    y_in = nc.declare_dram_parameter("y", [rows, NV], F32, isOutput=False)
    cf_in = nc.declare_dram_parameter("cf", [128, CF_W], F32, isOutput=False)
    ilp_in = nc.declare_dram_parameter("ilp", [128, n_groups * HD], F32, isOutput=False)
    out_ext = nc.declare_dram_parameter("out", [rows, NV], F32, isOutput=True)

    MUL = mybir.AluOpType.mult
    ADD = mybir.AluOpType.add
    SUB = mybir.AluOpType.subtract
    COPY = mybir.ActivationFunctionType.Copy

    pw0 = float(-KY / (2.0 * DV * _V[0]**3))
    pu0 = float(-KY / (DV * DV * _V[0]**2))
    vlast = float(_V[-1])

    with ExitStack() as ctx:
        tc = ctx.enter_context(tile.TileContext(nc))
        cpool = ctx.enter_context(tc.tile_pool(name="consts", bufs=1))

        cf = cpool.tile([128, CF_W], F32, tag="cf")
        nc.scalar.dma_start(cf[:, :], cf_in[:, :])
        ilp = cpool.tile([128, n_groups * HD], F32, tag="ilp")
        nc.gpsimd.dma_start(ilp[:, :], ilp_in[:, :])

        resetv = cf[:, 0:FD]
        reset1h = cf[:, FD:FD + HD]
        pw2kh = cf[:, FD + HD:FD + 2 * HD]
        g1wh = cf[:, FD + 2 * HD:FD + 3 * HD]
        g2wh = cf[:, FD + 3 * HD:FD + 4 * HD]

        # touch consts so the tile framework orders compute after the loads
        for nm, seg in (("tc_f", cf), ("tc_i", ilp)):
            tch = cpool.tile([128, 1], F32, tag=nm)
            nc.vector.tensor_copy(out=tch[:, :], in_=seg[:, 0:1])

        io = ctx.enter_context(tc.tile_pool(name="io", bufs=3))
        e1p = ctx.enter_context(tc.tile_pool(name="e1", bufs=2))
        wk = ctx.enter_context(tc.tile_pool(name="work", bufs=2))

        for g in range(n_groups):
            rsl = slice(g * GROUP_ROWS, (g + 1) * GROUP_ROWS)
            y_src = y_in[rsl, :].rearrange("(p j) v -> p (j v)", p=128)
            x_dst = out_ext[rsl, :].rearrange("(p j) v -> p (j v)", p=128)

            y4 = io.tile([128, FD], F32, tag="y4")
            nc.sync.dma_start(y4[:, :], y_src)
            y4v = y4[:, :].rearrange("p (j v) -> p j v", j=FUSE)

            # S1 per system via full-row ratio scan (E1_t = P(y*v)_t / v_t)
            E1 = e1p.tile([128, FD], F32, tag="E1")
            nc.vector.tensor_tensor_scan(E1[:, :], resetv, y4[:, :], 0.0,
                                         op0=MUL, op1=ADD)
            # S1 seeds, scaled for the half-unit weighted scans
            s1x2 = wk.tile([128, FUSE], F32, tag="s1x2")
            nc.scalar.activation(s1x2[:, :], E1[:, NV - 1::NV], COPY,
                                 bias=0.0, scale=pw0 * vlast)
            s1xp = wk.tile([128, FUSE], F32, tag="s1xp")
            nc.scalar.activation(s1xp[:, :], E1[:, NV - 1::NV], COPY,
                                 bias=0.0, scale=0.5 * pu0 * vlast)

            # head compaction
            yh = wk.tile([128, HD], F32, tag="yh")
            yhv = yh[:, :].rearrange("p (j v) -> p j v", j=FUSE)
            nc.gpsimd.tensor_copy(out=yhv[:, :, :], in_=y4v[:, :, 0:T0])

            wg1 = wk.tile([128, HD], F32, tag="wg1")
            nc.gpsimd.tensor_tensor(out=wg1[:, :], in0=yh[:, :], in1=g1wh, op=MUL)
            wg2 = wk.tile([128, HD], F32, tag="wg2")
            nc.gpsimd.tensor_tensor(out=wg2[:, :], in0=yh[:, :], in1=g2wh, op=MUL)
            # inject S1 seeds at each system's first head column
            nc.gpsimd.tensor_tensor(out=wg1[:, 0::T0], in0=wg1[:, 0::T0],
                                    in1=s1x2[:, :], op=ADD)
            nc.gpsimd.tensor_tensor(out=wg2[:, 0::T0], in0=wg2[:, 0::T0],
                                    in1=s1xp[:, :], op=ADD)

            # weighted ratio scans: wn = -w/2, un = -u/2
            wn = wk.tile([128, HD], F32, tag="wn")
            nc.vector.tensor_tensor_scan(wn[:, :], reset1h, wg1[:, :], 0.0,
                                         op0=MUL, op1=ADD)
            un = wk.tile([128, HD], F32, tag="un")
            nc.vector.tensor_tensor_scan(un[:, :], pw2kh, wg2[:, :], 0.0,
                                         op0=MUL, op1=ADD)

            # diagonal b = t3 + u/2 - il2*(2DV/v)*w
            t3 = wk.tile([128, HD], F32, tag="t3")
            nc.scalar.activation(t3[:, :], yh[:, :], COPY,
                                 bias=1.0, scale=float(8.0 * np.pi * Y_DT))
            b1 = wk.tile([128, HD], F32, tag="b1")
            nc.gpsimd.tensor_tensor(out=b1[:, :], in0=t3[:, :], in1=un[:, :], op=SUB)
            bil = wk.tile([128, HD], F32, tag="bil")
            nc.gpsimd.tensor_tensor(out=bil[:, :], in0=wn[:, :],
                                    in1=ilp[:, g * HD:(g + 1) * HD], op=MUL)
            bpos = wk.tile([128, HD], F32, tag="bpos")
            nc.gpsimd.tensor_tensor(out=bpos[:, :], in0=b1[:, :], in1=bil[:, :], op=ADD)
            binv = wk.tile([128, HD], F32, tag="binv")
            nc.vector.reciprocal(out=binv[:, :], in_=bpos[:, :])
            binv2 = wk.tile([128, HD], F32, tag="binv2")
            nc.scalar.activation(binv2[:, :], binv[:, :], COPY, bias=0.0, scale=2.0)

            # off-diagonals: a_n = -a/2, c_n = -c/2
            a_n = wk.tile([128, HD], F32, tag="a_n")
            nc.gpsimd.tensor_tensor(out=a_n[:, :], in0=un[:, :], in1=wn[:, :], op=SUB)
            c_n = wk.tile([128, HD], F32, tag="c_n")
            nc.gpsimd.tensor_tensor(out=c_n[:, :], in0=un[:, :], in1=wn[:, :], op=ADD)

            alpha = wk.tile([128, HD], F32, tag="alpha")    # -a/b
            nc.gpsimd.tensor_tensor(out=alpha[:, :], in0=a_n[:, :], in1=binv2[:, :], op=MUL)
            av = alpha[:, :].rearrange("p (j v) -> p j v", j=FUSE)
            nc.vector.memset(av[:, :, 0:1], 0.0)            # fwd scan reset
            mcp = wk.tile([128, HD], F32, tag="mcp")        # -c/b
            nc.gpsimd.tensor_tensor(out=mcp[:, :], in0=c_n[:, :], in1=binv2[:, :], op=MUL)
            mv = mcp[:, :].rearrange("p (j v) -> p j v", j=FUSE)
            nc.vector.memset(mv[:, :, T0 - 1:T0], 0.0)      # bwd scan reset
            beta = wk.tile([128, HD], F32, tag="beta")      # y/b
            nc.gpsimd.tensor_tensor(out=beta[:, :], in0=yh[:, :], in1=binv[:, :], op=MUL)

            dp = wk.tile([128, HD], F32, tag="dp")
            nc.vector.tensor_tensor_scan(dp[:, :], alpha[:, :], beta[:, :], 0.0,
                                         op0=MUL, op1=ADD)
            xh = wk.tile([128, HD], F32, tag="xh")
            nc.vector.tensor_tensor_scan(xh[:, ::-1], mcp[:, ::-1], dp[:, ::-1], 0.0,
                                         op0=MUL, op1=ADD)

            # scatter head solution into the input tile; tail passes through
            xhv = xh[:, :].rearrange("p (j v) -> p j v", j=FUSE)
            nc.gpsimd.tensor_copy(out=y4v[:, :, 0:T0], in_=xhv[:, :, :])
            if g % 3 == 2:
                nc.gpsimd.dma_start(x_dst, y4[:, :])
            else:
                nc.scalar.dma_start(x_dst, y4[:, :])

    if legalize:
        _legalize_multiwait(nc)
    return nc


_NC_CACHE = {}


def _get_nc(n_groups=N_GROUPS):
    if n_groups not in _NC_CACHE:
        _NC_CACHE[n_groups] = build_nc(n_groups)
    return _NC_CACHE[n_groups]


_CF_CACHE = None


def make_inputs(y_shard, il2_rows, n_groups=N_GROUPS):
    """Per-core input map. y_shard [rows, 512] f32; il2_rows [rows] f32
    (holding il*(il+1)/2 per row)."""
    global _CF_CACHE
    if _CF_CACHE is None:
        _CF_CACHE = np.broadcast_to(_profiles()[None, :], (128, CF_W)
                                    ).astype(np.float32).copy()
    il2 = il2_rows.reshape(n_groups, 128, FUSE)[:, :, 0]          # [g, 128]
    prof = (4.0 * DV / _V[:T0]).astype(np.float64)                # [T0]
    ilp = (il2[:, :, None, None] * prof[None, None, None, :])     # [g,128,1,T0]
    ilp = np.broadcast_to(ilp, (n_groups, 128, FUSE, T0))
    ilp = ilp.transpose(1, 0, 2, 3).reshape(128, n_groups * HD).astype(np.float32)
    return {
        "y": np.ascontiguousarray(y_shard, dtype=np.float32),
        "cf": _CF_CACHE,
        "ilp": np.ascontiguousarray(ilp),
    }


def kernel(y, il_arr):
    y = np.asarray(y, dtype=np.float32)
    il_arr = np.asarray(il_arr)
    yf = y.reshape(ROWS_TOTAL, NV)
    il_f = il_arr.astype(np.float64)
    il2_all = np.repeat(il_f * (il_f + 1.0) / 2.0, NX * NY).astype(np.float32)

    nc = _get_nc()
    in_maps = []
    for c in range(N_CORES):
        rs = slice(c * ROWS_PER_CORE, (c + 1) * ROWS_PER_CORE)
        in_maps.append(make_inputs(yf[rs], il2_all[rs]))
    res = run_bass_kernel_spmd(nc, in_maps, core_ids=list(range(N_CORES)))
    outs = [res.results[c]["out"] for c in range(N_CORES)]
    x = np.concatenate(outs, axis=0).reshape(N_MODES, NX, NY, NV)
    return x.astype(np.float32)


# revision 5
# speedup vs baseline: 6.3199x; 1.1283x over previous
"""Anisotropic collisions kernel for 8 TRN2 NeuronCores.

Math: for each of 9*64*64 = 36864 independent systems (mode, spatial cell),
build tridiagonal coefficients from Rosenbluth cumulative integrals of
flm(v) along v (512 points), then solve the tridiagonal system along v.

Key structural facts exploited (validated numerically vs f64 Thomas):
  1. The collision coefficients u (c2-term) and w (c1-term) decay ~1/v^2;
     beyond v-index T0 the tridiagonal system is identity to ~1e-4 * x.
     The solve therefore runs only on the first T0 columns of each
     512-system ("head"); the tail passes through (x = y) via an in-place
     scatter of the head solution into the input tile followed by one
     contiguous output DMA. Only S1 = sum(y*v) needs the full row: one
     full-length ratio scan (E1) on DVE.
  2. Thomas without the cp refinement (cp = c/b) is accurate to ~3e-3.

Scheduling: input DMA rides the SP queue, output DMA the Pool queue
(transfers on different queues overlap in time). Scans + reciprocal are
DVE-only ops; every elementwise tensor_tensor runs on the Pool engine
(flat-rate ALU, otherwise idle); activations (scaled copies) run on ACT.
Scale factors are folded into host-precomputed profiles so no
tensor_scalar / scalar_tensor_tensor is needed (TensorScalarPtr is
DVE-only on this toolchain): the weighted scans emit -w/2 and -u/2
directly, and the il2*(2DV/v) diagonal term uses a per-group outer
product profile il2[p] * 4DV/v[f].

Toolchain notes: this walrus build accepts only ONE sync-wait per
instruction; multi-wait instructions are split into standalone
InstEventSemaphore waits in a post-pass.
"""

import numpy as np
from contextlib import ExitStack

import concourse.bass as bass
import concourse.tile as tile
import concourse.mybir as mybir
from concourse.bass_utils import run_bass_kernel_spmd

F32 = mybir.dt.float32

NX, NY, NV = 64, 64, 512
N_MODES = 9
DV = 0.015625
Y_DT = 1.0e-12
FOUR_PI = 4.0 * np.pi
KY = FOUR_PI * Y_DT / 3.0

N_CORES = 8
ROWS_TOTAL = N_MODES * NX * NY            # 36864
ROWS_PER_CORE = ROWS_TOTAL // N_CORES     # 4608
FUSE = 4                                  # systems per partition row
GROUP_ROWS = 128 * FUSE                   # 512 systems per group
N_GROUPS = ROWS_PER_CORE // GROUP_ROWS    # 9
FD = FUSE * NV                            # 2048
T0 = 32                                   # head length per system
HD = FUSE * T0

_V = (np.arange(NV, dtype=np.float64) + 1.0) * DV

# f32 const blob: resetv [FD], then reset1h, pw2kh, g1wh, g2wh [HD each]
CF_W = FD + 4 * HD


def _profiles():
    v = _V
    vh = v[:T0]
    g1 = 3.0 * v**2 - v**4 - 2.0 * v
    g2 = v**4 - v
    pwn = -KY / (2.0 * DV * v**3)         # wn' = -w/2  (0.5 folded in)
    pun = -KY / (DV * DV * v**2)          # un' = -u/2
    r1 = np.ones(NV)
    r1[1:] = v[:-1] / v[1:]
    r1[0] = 0.0                           # E1 reset at each system start
    r3 = np.ones(T0)
    r3[1:] = (vh[:-1] / vh[1:])**3
    r3[0] = 0.0
    r2 = np.ones(T0)
    r2[1:] = (vh[:-1] / vh[1:])**2
    r2[0] = 0.0
    return np.concatenate([
        np.tile(r1, FUSE),
        np.tile(r3, FUSE),
        np.tile(r2, FUSE),
        np.tile(0.5 * g1[:T0] * pwn[:T0], FUSE),
        np.tile(0.5 * g2[:T0] * pun[:T0], FUSE),
    ])


def _legalize_multiwait(nc):
    """Split instructions with >1 sync wait: keep one wait on the
    instruction, hoist the rest onto standalone InstEventSemaphore ops
    immediately before it on the same engine (this walrus accepts only one
    wait per instruction)."""
    n = [0]

    def fresh(engine, wait):
        n[0] += 1
        return mybir.InstEventSemaphore(
            name=f"mwsplit-{n[0]}",
            engine=engine,
            sync_info=mybir.SyncInfo(on_wait=[wait], on_update=[]),
        )

    for fn in nc.m.functions:
        for blk in fn.blocks:
            out = []
            for ins in blk.instructions:
                si = ins.sync_info
                if si is not None and si.on_wait is not None and len(si.on_wait) > 1:
                    waits = list(si.on_wait)
                    for w in waits[:-1]:
                        out.append(fresh(ins.engine, w))
                    si.on_wait = [waits[-1]]
                out.append(ins)
            blk.instructions[:] = out


def build_nc(n_groups=N_GROUPS, legalize=True):
    nc = bass.Bass()
    rows = n_groups * GROUP_ROWS
    y_in = nc.declare_dram_parameter("y", [rows, NV], F32, isOutput=False)
    cf_in = nc.declare_dram_parameter("cf", [128, CF_W], F32, isOutput=False)
    ilp_in = nc.declare_dram_parameter("ilp", [128, n_groups * HD], F32, isOutput=False)
    out_ext = nc.declare_dram_parameter("out", [rows, NV], F32, isOutput=True)

    MUL = mybir.AluOpType.mult
    ADD = mybir.AluOpType.add
    SUB = mybir.AluOpType.subtract
    COPY = mybir.ActivationFunctionType.Copy

    pw0 = float(-KY / (2.0 * DV * _V[0]**3))
    pu0 = float(-KY / (DV * DV * _V[0]**2))
    vlast = float(_V[-1])

    with ExitStack() as ctx:
        tc = ctx.enter_context(tile.TileContext(nc))
        cpool = ctx.enter_context(tc.tile_pool(name="consts", bufs=1))

        cf = cpool.tile([128, CF_W], F32, tag="cf")
        nc.scalar.dma_start(cf[:, :], cf_in[:, :])
        ilp = cpool.tile([128, n_groups * HD], F32, tag="ilp")
        nc.gpsimd.dma_start(ilp[:, :], ilp_in[:, :])

        resetv = cf[:, 0:FD]
        reset1h = cf[:, FD:FD + HD]
        pw2kh = cf[:, FD + HD:FD + 2 * HD]
        g1wh = cf[:, FD + 2 * HD:FD + 3 * HD]
        g2wh = cf[:, FD + 3 * HD:FD + 4 * HD]

        # touch consts so the tile framework orders compute after the loads
        for nm, seg in (("tc_f", cf), ("tc_i", ilp)):
            tch = cpool.tile([128, 1], F32, tag=nm)
            nc.vector.tensor_copy(out=tch[:, :], in_=seg[:, 0:1])

        io = ctx.enter_context(tc.tile_pool(name="io", bufs=4))
        e1p = ctx.enter_context(tc.tile_pool(name="e1", bufs=2))
        wk = ctx.enter_context(tc.tile_pool(name="work", bufs=3))

        # --- 3-stage software pipeline -------------------------------
        # A(g): input DMA, E1 scan, S1 seeds, head compaction, weighted
        #       scans wn/un, t3.
        # B(g): diagonal/off-diagonal assembly, reciprocal, alpha/mcp/beta.
        # C(g): dp/xb solve scans, scatter, output DMA.
        # Issuing A(g), C(g-2), B(g-1) keeps every in-order engine queue
        # stocked with ready work (the g-2 solve depends only on stage-B
        # results from the previous iteration).
        st = {}

        def stage_a(g):
            rsl = slice(g * GROUP_ROWS, (g + 1) * GROUP_ROWS)
            y_src = y_in[rsl, :].rearrange("(p j) v -> p (j v)", p=128)
            d = {}
            y4 = io.tile([128, FD], F32, tag="y4")
            nc.sync.dma_start(y4[:, :], y_src)
            d["y4"] = y4
            y4v = y4[:, :].rearrange("p (j v) -> p j v", j=FUSE)
            d["y4v"] = y4v

            # S1 per system via full-row ratio scan (E1_t = P(y*v)_t / v_t)
            E1 = e1p.tile([128, FD], F32, tag="E1")
            nc.vector.tensor_tensor_scan(E1[:, :], resetv, y4[:, :], 0.0,
                                         op0=MUL, op1=ADD)
            s1x2 = wk.tile([128, FUSE], F32, tag="s1x2")
            nc.scalar.activation(s1x2[:, :], E1[:, NV - 1::NV], COPY,
                                 bias=0.0, scale=pw0 * vlast)
            s1xp = wk.tile([128, FUSE], F32, tag="s1xp")
            nc.scalar.activation(s1xp[:, :], E1[:, NV - 1::NV], COPY,
                                 bias=0.0, scale=0.5 * pu0 * vlast)

            yh = wk.tile([128, HD], F32, tag="yh")
            yhv = yh[:, :].rearrange("p (j v) -> p j v", j=FUSE)
            nc.gpsimd.tensor_copy(out=yhv[:, :, :], in_=y4v[:, :, 0:T0])
            d["yh"] = yh

            wg1 = wk.tile([128, HD], F32, tag="wg1")
            nc.gpsimd.tensor_tensor(out=wg1[:, :], in0=yh[:, :], in1=g1wh, op=MUL)
            wg2 = wk.tile([128, HD], F32, tag="wg2")
            nc.gpsimd.tensor_tensor(out=wg2[:, :], in0=yh[:, :], in1=g2wh, op=MUL)
            nc.gpsimd.tensor_tensor(out=wg1[:, 0::T0], in0=wg1[:, 0::T0],
                                    in1=s1x2[:, :], op=ADD)
            nc.gpsimd.tensor_tensor(out=wg2[:, 0::T0], in0=wg2[:, 0::T0],
                                    in1=s1xp[:, :], op=ADD)

            wn = wk.tile([128, HD], F32, tag="wn")
            nc.vector.tensor_tensor_scan(wn[:, :], reset1h, wg1[:, :], 0.0,
                                         op0=MUL, op1=ADD)
            un = wk.tile([128, HD], F32, tag="un")
            nc.vector.tensor_tensor_scan(un[:, :], pw2kh, wg2[:, :], 0.0,
                                         op0=MUL, op1=ADD)
            d["wn"], d["un"] = wn, un

            t3 = wk.tile([128, HD], F32, tag="t3")
            nc.scalar.activation(t3[:, :], yh[:, :], COPY,
                                 bias=1.0, scale=float(8.0 * np.pi * Y_DT))
            d["t3"] = t3
            return d

        def stage_b(g, d):
            wn, un, yh, t3 = d["wn"], d["un"], d["yh"], d["t3"]
            b1 = wk.tile([128, HD], F32, tag="b1")
            nc.gpsimd.tensor_tensor(out=b1[:, :], in0=t3[:, :], in1=un[:, :], op=SUB)
            bil = wk.tile([128, HD], F32, tag="bil")
            nc.gpsimd.tensor_tensor(out=bil[:, :], in0=wn[:, :],
                                    in1=ilp[:, g * HD:(g + 1) * HD], op=MUL)
            bpos = wk.tile([128, HD], F32, tag="bpos")
            nc.gpsimd.tensor_tensor(out=bpos[:, :], in0=b1[:, :], in1=bil[:, :], op=ADD)
            binv = wk.tile([128, HD], F32, tag="binv")
            nc.vector.reciprocal(out=binv[:, :], in_=bpos[:, :])
            binv2 = wk.tile([128, HD], F32, tag="binv2")
            nc.scalar.activation(binv2[:, :], binv[:, :], COPY, bias=0.0, scale=2.0)

            a_n = wk.tile([128, HD], F32, tag="a_n")
            nc.gpsimd.tensor_tensor(out=a_n[:, :], in0=un[:, :], in1=wn[:, :], op=SUB)
            c_n = wk.tile([128, HD], F32, tag="c_n")
            nc.gpsimd.tensor_tensor(out=c_n[:, :], in0=un[:, :], in1=wn[:, :], op=ADD)

            alpha = wk.tile([128, HD], F32, tag="alpha")    # -a/b
            nc.gpsimd.tensor_tensor(out=alpha[:, :], in0=a_n[:, :], in1=binv2[:, :], op=MUL)
            av = alpha[:, :].rearrange("p (j v) -> p j v", j=FUSE)
            nc.vector.memset(av[:, :, 0:1], 0.0)            # fwd scan reset
            mcp = wk.tile([128, HD], F32, tag="mcp")        # -c/b
            nc.gpsimd.tensor_tensor(out=mcp[:, :], in0=c_n[:, :], in1=binv2[:, :], op=MUL)
            mv = mcp[:, :].rearrange("p (j v) -> p j v", j=FUSE)
            nc.vector.memset(mv[:, :, T0 - 1:T0], 0.0)      # bwd scan reset
            beta = wk.tile([128, HD], F32, tag="beta")      # y/b
            nc.gpsimd.tensor_tensor(out=beta[:, :], in0=yh[:, :], in1=binv[:, :], op=MUL)
            d["alpha"], d["mcp"], d["beta"] = alpha, mcp, beta

        def stage_c(g, d):
            rsl = slice(g * GROUP_ROWS, (g + 1) * GROUP_ROWS)
            x_dst = out_ext[rsl, :].rearrange("(p j) v -> p (j v)", p=128)
            alpha, mcp, beta = d["alpha"], d["mcp"], d["beta"]
            dp = wk.tile([128, HD], F32, tag="dp")
            nc.vector.tensor_tensor_scan(dp[:, :], alpha[:, :], beta[:, :], 0.0,
                                         op0=MUL, op1=ADD)
            xh = wk.tile([128, HD], F32, tag="xh")
            nc.vector.tensor_tensor_scan(xh[:, ::-1], mcp[:, ::-1], dp[:, ::-1], 0.0,
                                         op0=MUL, op1=ADD)
            xhv = xh[:, :].rearrange("p (j v) -> p j v", j=FUSE)
            nc.gpsimd.tensor_copy(out=d["y4v"][:, :, 0:T0], in_=xhv[:, :, :])
            if g % 3 == 2:
                nc.gpsimd.dma_start(x_dst, d["y4"][:, :])
            else:
                nc.scalar.dma_start(x_dst, d["y4"][:, :])

        for g in range(n_groups + 2):
            if g < n_groups:
                st[g] = stage_a(g)
            if g - 2 >= 0:
                stage_c(g - 2, st[g - 2])
                del st[g - 2]
            if g - 1 < n_groups and g - 1 >= 0:
                stage_b(g - 1, st[g - 1])

    if legalize:
        _legalize_multiwait(nc)
    return nc


_NC_CACHE = {}


def _get_nc(n_groups=N_GROUPS):
    if n_groups not in _NC_CACHE:
        _NC_CACHE[n_groups] = build_nc(n_groups)
    return _NC_CACHE[n_groups]


_CF_CACHE = None


def make_inputs(y_shard, il2_rows, n_groups=N_GROUPS):
    """Per-core input map. y_shard [rows, 512] f32; il2_rows [rows] f32
    (holding il*(il+1)/2 per row)."""
    global _CF_CACHE
    if _CF_CACHE is None:
        _CF_CACHE = np.broadcast_to(_profiles()[None, :], (128, CF_W)
                                    ).astype(np.float32).copy()
    il2 = il2_rows.reshape(n_groups, 128, FUSE)[:, :, 0]          # [g, 128]
    prof = (4.0 * DV / _V[:T0]).astype(np.float64)                # [T0]
    ilp = (il2[:, :, None, None] * prof[None, None, None, :])     # [g,128,1,T0]
    ilp = np.broadcast_to(ilp, (n_groups, 128, FUSE, T0))
    ilp = ilp.transpose(1, 0, 2, 3).reshape(128, n_groups * HD).astype(np.float32)
    return {
        "y": np.ascontiguousarray(y_shard, dtype=np.float32),
        "cf": _CF_CACHE,
        "ilp": np.ascontiguousarray(ilp),
    }


def kernel(y, il_arr):
    y = np.asarray(y, dtype=np.float32)
    il_arr = np.asarray(il_arr)
    yf = y.reshape(ROWS_TOTAL, NV)
    il_f = il_arr.astype(np.float64)
    il2_all = np.repeat(il_f * (il_f + 1.0) / 2.0, NX * NY).astype(np.float32)

    nc = _get_nc()
    in_maps = []
    for c in range(N_CORES):
        rs = slice(c * ROWS_PER_CORE, (c + 1) * ROWS_PER_CORE)
        in_maps.append(make_inputs(yf[rs], il2_all[rs]))
    res = run_bass_kernel_spmd(nc, in_maps, core_ids=list(range(N_CORES)))
    outs = [res.results[c]["out"] for c in range(N_CORES)]
    x = np.concatenate(outs, axis=0).reshape(N_MODES, NX, NY, NV)
    return x.astype(np.float32)


# revision 6
# speedup vs baseline: 7.8678x; 1.2449x over previous
"""Anisotropic collisions kernel for 8 TRN2 NeuronCores.

Math: for each of 9*64*64 = 36864 independent systems (mode, spatial cell),
build tridiagonal coefficients from Rosenbluth cumulative integrals of
flm(v) along v (512 points), then solve the tridiagonal system along v.

Key structural facts exploited (validated numerically vs f64 Thomas):
  1. The collision coefficients u (c2-term) and w (c1-term) decay ~1/v^2;
     beyond v-index T0 the tridiagonal system is identity to ~1e-4 * x.
     The solve therefore runs only on the first T0 columns of each
     512-system ("head"); the tail passes through (x = y) via an in-place
     scatter of the head solution into the input tile followed by one
     contiguous output DMA. Only S1 = sum(y*v) needs the full row: one
     full-length ratio scan (E1) on DVE.
  2. Thomas without the cp refinement (cp = c/b) is accurate to ~3e-3.

Scheduling: input DMA rides the SP queue, output DMA the Pool queue
(transfers on different queues overlap in time). Scans + reciprocal are
DVE-only ops; every elementwise tensor_tensor runs on the Pool engine
(flat-rate ALU, otherwise idle); activations (scaled copies) run on ACT.
Scale factors are folded into host-precomputed profiles so no
tensor_scalar / scalar_tensor_tensor is needed (TensorScalarPtr is
DVE-only on this toolchain): the weighted scans emit -w/2 and -u/2
directly, and the il2*(2DV/v) diagonal term uses a per-group outer
product profile il2[p] * 4DV/v[f].

Toolchain notes: this walrus build accepts only ONE sync-wait per
instruction; multi-wait instructions are split into standalone
InstEventSemaphore waits in a post-pass.
"""

import numpy as np
from contextlib import ExitStack

import concourse.bass as bass
import concourse.tile as tile
import concourse.mybir as mybir
from concourse.bass_utils import run_bass_kernel_spmd

F32 = mybir.dt.float32

NX, NY, NV = 64, 64, 512
N_MODES = 9
DV = 0.015625
Y_DT = 1.0e-12
FOUR_PI = 4.0 * np.pi
KY = FOUR_PI * Y_DT / 3.0

N_CORES = 8
ROWS_TOTAL = N_MODES * NX * NY            # 36864
ROWS_PER_CORE = ROWS_TOTAL // N_CORES     # 4608
FUSE = 4                                  # systems per partition row
GROUP_ROWS = 128 * FUSE                   # 512 systems per group
N_GROUPS = ROWS_PER_CORE // GROUP_ROWS    # 9
FD = FUSE * NV                            # 2048
T0 = 32                                   # head length per system
HD = FUSE * T0

_V = (np.arange(NV, dtype=np.float64) + 1.0) * DV

# f32 const blob: resetv [FD], then reset1h, pw2kh, g1wh, g2wh [HD each]
CF_W = FD + 4 * HD


def _profiles():
    v = _V
    vh = v[:T0]
    g1 = 3.0 * v**2 - v**4 - 2.0 * v
    g2 = v**4 - v
    pwn = -KY / (2.0 * DV * v**3)         # wn' = -w/2  (0.5 folded in)
    pun = -KY / (DV * DV * v**2)          # un' = -u/2
    r1 = np.ones(NV)
    r1[1:] = v[:-1] / v[1:]
    r1[0] = 0.0                           # E1 reset at each system start
    r3 = np.ones(T0)
    r3[1:] = (vh[:-1] / vh[1:])**3
    r3[0] = 0.0
    r2 = np.ones(T0)
    r2[1:] = (vh[:-1] / vh[1:])**2
    r2[0] = 0.0
    return np.concatenate([
        np.tile(r1, FUSE),
        np.tile(r3, FUSE),
        np.tile(r2, FUSE),
        np.tile(0.5 * g1[:T0] * pwn[:T0], FUSE),
        np.tile(0.5 * g2[:T0] * pun[:T0], FUSE),
    ])


def _legalize_multiwait(nc):
    """Split instructions with >1 sync wait: keep one wait on the
    instruction, hoist the rest onto standalone InstEventSemaphore ops
    immediately before it on the same engine (this walrus accepts only one
    wait per instruction)."""
    n = [0]

    def fresh(engine, wait):
        n[0] += 1
        return mybir.InstEventSemaphore(
            name=f"mwsplit-{n[0]}",
            engine=engine,
            sync_info=mybir.SyncInfo(on_wait=[wait], on_update=[]),
        )

    for fn in nc.m.functions:
        for blk in fn.blocks:
            out = []
            for ins in blk.instructions:
                si = ins.sync_info
                if si is not None and si.on_wait is not None and len(si.on_wait) > 1:
                    waits = list(si.on_wait)
                    for w in waits[:-1]:
                        out.append(fresh(ins.engine, w))
                    si.on_wait = [waits[-1]]
                out.append(ins)
            blk.instructions[:] = out


def build_nc(n_groups=N_GROUPS, legalize=True):
    nc = bass.Bass()
    rows = n_groups * GROUP_ROWS
    y_in = nc.declare_dram_parameter("y", [rows, NV], F32, isOutput=False)
    cf_in = nc.declare_dram_parameter("cf", [128, CF_W], F32, isOutput=False)
    ilp_in = nc.declare_dram_parameter("ilp", [128, n_groups * HD], F32, isOutput=False)
    out_ext = nc.declare_dram_parameter("out", [rows, NV], F32, isOutput=True)

    MUL = mybir.AluOpType.mult
    ADD = mybir.AluOpType.add
    SUB = mybir.AluOpType.subtract
    COPY = mybir.ActivationFunctionType.Copy

    pw0 = float(-KY / (2.0 * DV * _V[0]**3))
    pu0 = float(-KY / (DV * DV * _V[0]**2))
    vlast = float(_V[-1])

    with ExitStack() as ctx:
        tc = ctx.enter_context(tile.TileContext(nc))
        cpool = ctx.enter_context(tc.tile_pool(name="consts", bufs=1))

        cf = cpool.tile([128, CF_W], F32, tag="cf")
        nc.gpsimd.dma_start(cf[:, :], cf_in[:, :])
        ilp = cpool.tile([128, n_groups * HD], F32, tag="ilp")
        nc.gpsimd.dma_start(ilp[:, :], ilp_in[:, :])

        resetv = cf[:, 0:FD]
        reset1h = cf[:, FD:FD + HD]
        pw2kh = cf[:, FD + HD:FD + 2 * HD]
        g1wh = cf[:, FD + 2 * HD:FD + 3 * HD]
        g2wh = cf[:, FD + 3 * HD:FD + 4 * HD]

        twos = cpool.tile([128, HD], F32, tag="twos")
        nc.gpsimd.memset(twos[:, :], 2.0)

        # touch consts so the tile framework orders compute after the loads
        for nm, seg in (("tc_f", cf), ("tc_i", ilp)):
            tch = cpool.tile([128, 1], F32, tag=nm)
            nc.vector.tensor_copy(out=tch[:, :], in_=seg[:, 0:1])

        io = ctx.enter_context(tc.tile_pool(name="io", bufs=6))
        e1p = ctx.enter_context(tc.tile_pool(name="e1", bufs=2))
        wk = ctx.enter_context(tc.tile_pool(name="work", bufs=3))

        # --- 3-stage software pipeline -------------------------------
        # A(g): input DMA, E1 scan, S1 seeds, head compaction, weighted
        #       scans wn/un, t3.
        # B(g): diagonal/off-diagonal assembly, reciprocal, alpha/mcp/beta.
        # C(g): dp/xb solve scans, scatter, output DMA.
        # Issuing A(g), C(g-2), B(g-1) keeps every in-order engine queue
        # stocked with ready work (the g-2 solve depends only on stage-B
        # results from the previous iteration).
        st = {}

        def stage_a(g):
            rsl = slice(g * GROUP_ROWS, (g + 1) * GROUP_ROWS)
            y_src = y_in[rsl, :].rearrange("(p j) v -> p (j v)", p=128)
            d = {}
            y4 = io.tile([128, FD], F32, tag="y4")
            nc.sync.dma_start(y4[:, :], y_src)
            d["y4"] = y4
            y4v = y4[:, :].rearrange("p (j v) -> p j v", j=FUSE)
            d["y4v"] = y4v

            # S1 per system via full-row ratio scan (E1_t = P(y*v)_t / v_t)
            E1 = e1p.tile([128, FD], F32, tag="E1")
            nc.vector.tensor_tensor_scan(E1[:, :], resetv, y4[:, :], 0.0,
                                         op0=MUL, op1=ADD)
            s1x2 = wk.tile([128, FUSE], F32, tag="s1x2")
            nc.scalar.activation(s1x2[:, :], E1[:, NV - 1::NV], COPY,
                                 bias=0.0, scale=pw0 * vlast)
            s1xp = wk.tile([128, FUSE], F32, tag="s1xp")
            nc.scalar.activation(s1xp[:, :], E1[:, NV - 1::NV], COPY,
                                 bias=0.0, scale=0.5 * pu0 * vlast)

            yh = wk.tile([128, HD], F32, tag="yh")
            yhv = yh[:, :].rearrange("p (j v) -> p j v", j=FUSE)
            nc.gpsimd.tensor_copy(out=yhv[:, :, :], in_=y4v[:, :, 0:T0])
            d["yh"] = yh

            wg1 = wk.tile([128, HD], F32, tag="wg1")
            nc.gpsimd.tensor_tensor(out=wg1[:, :], in0=yh[:, :], in1=g1wh, op=MUL)
            wg2 = wk.tile([128, HD], F32, tag="wg2")
            nc.gpsimd.tensor_tensor(out=wg2[:, :], in0=yh[:, :], in1=g2wh, op=MUL)
            nc.gpsimd.tensor_tensor(out=wg1[:, 0::T0], in0=wg1[:, 0::T0],
                                    in1=s1x2[:, :], op=ADD)
            nc.gpsimd.tensor_tensor(out=wg2[:, 0::T0], in0=wg2[:, 0::T0],
                                    in1=s1xp[:, :], op=ADD)

            wn = wk.tile([128, HD], F32, tag="wn")
            nc.vector.tensor_tensor_scan(wn[:, :], reset1h, wg1[:, :], 0.0,
                                         op0=MUL, op1=ADD)
            un = wk.tile([128, HD], F32, tag="un")
            nc.vector.tensor_tensor_scan(un[:, :], pw2kh, wg2[:, :], 0.0,
                                         op0=MUL, op1=ADD)
            d["wn"], d["un"] = wn, un

            t3 = wk.tile([128, HD], F32, tag="t3")
            nc.scalar.activation(t3[:, :], yh[:, :], COPY,
                                 bias=1.0, scale=float(8.0 * np.pi * Y_DT))
            d["t3"] = t3
            return d

        def stage_b(g, d):
            wn, un, yh, t3 = d["wn"], d["un"], d["yh"], d["t3"]
            b1 = wk.tile([128, HD], F32, tag="b1")
            nc.gpsimd.tensor_tensor(out=b1[:, :], in0=t3[:, :], in1=un[:, :], op=SUB)
            bil = wk.tile([128, HD], F32, tag="bil")
            nc.gpsimd.tensor_tensor(out=bil[:, :], in0=wn[:, :],
                                    in1=ilp[:, g * HD:(g + 1) * HD], op=MUL)
            bpos = wk.tile([128, HD], F32, tag="bpos")
            nc.gpsimd.tensor_tensor(out=bpos[:, :], in0=b1[:, :], in1=bil[:, :], op=ADD)
            binv = wk.tile([128, HD], F32, tag="binv")
            nc.vector.reciprocal(out=binv[:, :], in_=bpos[:, :])
            binv2 = wk.tile([128, HD], F32, tag="binv2")
            nc.gpsimd.tensor_tensor(out=binv2[:, :], in0=binv[:, :], in1=twos[:, :], op=MUL)

            a_n = wk.tile([128, HD], F32, tag="a_n")
            nc.gpsimd.tensor_tensor(out=a_n[:, :], in0=un[:, :], in1=wn[:, :], op=SUB)
            c_n = wk.tile([128, HD], F32, tag="c_n")
            nc.gpsimd.tensor_tensor(out=c_n[:, :], in0=un[:, :], in1=wn[:, :], op=ADD)

            alpha = wk.tile([128, HD], F32, tag="alpha")    # -a/b
            nc.gpsimd.tensor_tensor(out=alpha[:, :], in0=a_n[:, :], in1=binv2[:, :], op=MUL)
            av = alpha[:, :].rearrange("p (j v) -> p j v", j=FUSE)
            nc.gpsimd.memset(av[:, :, 0:1], 0.0)            # fwd scan reset
            mcp = wk.tile([128, HD], F32, tag="mcp")        # -c/b
            nc.gpsimd.tensor_tensor(out=mcp[:, :], in0=c_n[:, :], in1=binv2[:, :], op=MUL)
            mv = mcp[:, :].rearrange("p (j v) -> p j v", j=FUSE)
            nc.gpsimd.memset(mv[:, :, T0 - 1:T0], 0.0)      # bwd scan reset
            beta = wk.tile([128, HD], F32, tag="beta")      # y/b
            nc.gpsimd.tensor_tensor(out=beta[:, :], in0=yh[:, :], in1=binv[:, :], op=MUL)
            d["alpha"], d["mcp"], d["beta"] = alpha, mcp, beta

        def stage_c(g, d):
            rsl = slice(g * GROUP_ROWS, (g + 1) * GROUP_ROWS)
            x_dst = out_ext[rsl, :].rearrange("(p j) v -> p (j v)", p=128)
            alpha, mcp, beta = d["alpha"], d["mcp"], d["beta"]
            dp = wk.tile([128, HD], F32, tag="dp")
            nc.vector.tensor_tensor_scan(dp[:, :], alpha[:, :], beta[:, :], 0.0,
                                         op0=MUL, op1=ADD)
            xh = wk.tile([128, HD], F32, tag="xh")
            nc.vector.tensor_tensor_scan(xh[:, ::-1], mcp[:, ::-1], dp[:, ::-1], 0.0,
                                         op0=MUL, op1=ADD)
            xhv = xh[:, :].rearrange("p (j v) -> p j v", j=FUSE)
            nc.gpsimd.tensor_copy(out=d["y4v"][:, :, 0:T0], in_=xhv[:, :, :])
            if g >= n_groups - 2:
                h = FD // 2
                nc.sync.dma_start(x_dst[:, 0:h], d["y4"][:, 0:h])
                eng = nc.scalar if g % 2 == 0 else nc.gpsimd
                eng.dma_start(x_dst[:, h:FD], d["y4"][:, h:FD])
            elif g % 2 == 1:
                nc.gpsimd.dma_start(x_dst, d["y4"][:, :])
            else:
                nc.scalar.dma_start(x_dst, d["y4"][:, :])

        for g in range(n_groups + 2):
            if g < n_groups:
                st[g] = stage_a(g)
            if g - 2 >= 0:
                stage_c(g - 2, st[g - 2])
                del st[g - 2]
            if g - 1 < n_groups and g - 1 >= 0:
                stage_b(g - 1, st[g - 1])

    if legalize:
        _legalize_multiwait(nc)
    return nc


_NC_CACHE = {}


def _get_nc(n_groups=N_GROUPS):
    if n_groups not in _NC_CACHE:
        _NC_CACHE[n_groups] = build_nc(n_groups)
    return _NC_CACHE[n_groups]


_CF_CACHE = None


def make_inputs(y_shard, il2_rows, n_groups=N_GROUPS):
    """Per-core input map. y_shard [rows, 512] f32; il2_rows [rows] f32
    (holding il*(il+1)/2 per row)."""
    global _CF_CACHE
    if _CF_CACHE is None:
        _CF_CACHE = np.broadcast_to(_profiles()[None, :], (128, CF_W)
                                    ).astype(np.float32).copy()
    il2 = il2_rows.reshape(n_groups, 128, FUSE)[:, :, 0]          # [g, 128]
    prof = (4.0 * DV / _V[:T0]).astype(np.float64)                # [T0]
    ilp = (il2[:, :, None, None] * prof[None, None, None, :])     # [g,128,1,T0]
    ilp = np.broadcast_to(ilp, (n_groups, 128, FUSE, T0))
    ilp = ilp.transpose(1, 0, 2, 3).reshape(128, n_groups * HD).astype(np.float32)
    return {
        "y": np.ascontiguousarray(y_shard, dtype=np.float32),
        "cf": _CF_CACHE,
        "ilp": np.ascontiguousarray(ilp),
    }


def kernel(y, il_arr):
    y = np.asarray(y, dtype=np.float32)
    il_arr = np.asarray(il_arr)
    yf = y.reshape(ROWS_TOTAL, NV)
    il_f = il_arr.astype(np.float64)
    il2_all = np.repeat(il_f * (il_f + 1.0) / 2.0, NX * NY).astype(np.float32)

    nc = _get_nc()
    in_maps = []
    for c in range(N_CORES):
        rs = slice(c * ROWS_PER_CORE, (c + 1) * ROWS_PER_CORE)
        in_maps.append(make_inputs(yf[rs], il2_all[rs]))
    res = run_bass_kernel_spmd(nc, in_maps, core_ids=list(range(N_CORES)))
    outs = [res.results[c]["out"] for c in range(N_CORES)]
    x = np.concatenate(outs, axis=0).reshape(N_MODES, NX, NY, NV)
    return x.astype(np.float32)


# revision 7
# speedup vs baseline: 8.4601x; 1.0753x over previous
"""Anisotropic collisions kernel for 8 TRN2 NeuronCores.

Math: for each of 9*64*64 = 36864 independent systems (mode, spatial cell),
build tridiagonal coefficients from Rosenbluth cumulative integrals of
flm(v) along v (512 points), then solve the tridiagonal system along v.

Key structural facts exploited (validated numerically vs f64 Thomas):
  1. The collision coefficients u (c2-term) and w (c1-term) decay ~1/v^2;
     beyond v-index T0 the tridiagonal system is identity to ~1e-4 * x.
     The solve therefore runs only on the first T0 columns of each
     512-system ("head"); the tail passes through (x = y) via an in-place
     scatter of the head solution into the input tile followed by one
     contiguous output DMA. Only S1 = sum(y*v) needs the full row: one
     full-length ratio scan (E1) on DVE.
  2. Thomas without the cp refinement (cp = c/b) is accurate to ~3e-3.

Scheduling: input DMA rides the SP queue, output DMA the Pool queue
(transfers on different queues overlap in time). Scans + reciprocal are
DVE-only ops; every elementwise tensor_tensor runs on the Pool engine
(flat-rate ALU, otherwise idle); activations (scaled copies) run on ACT.
Scale factors are folded into host-precomputed profiles so no
tensor_scalar / scalar_tensor_tensor is needed (TensorScalarPtr is
DVE-only on this toolchain): the weighted scans emit -w/2 and -u/2
directly, and the il2*(2DV/v) diagonal term uses a per-group outer
product profile il2[p] * 4DV/v[f].

Toolchain notes: this walrus build accepts only ONE sync-wait per
instruction; multi-wait instructions are split into standalone
InstEventSemaphore waits in a post-pass.
"""

import numpy as np
from contextlib import ExitStack

import concourse.bass as bass
import concourse.tile as tile
import concourse.mybir as mybir
from concourse.bass_utils import run_bass_kernel_spmd

F32 = mybir.dt.float32

NX, NY, NV = 64, 64, 512
N_MODES = 9
DV = 0.015625
Y_DT = 1.0e-12
FOUR_PI = 4.0 * np.pi
KY = FOUR_PI * Y_DT / 3.0

N_CORES = 8
ROWS_TOTAL = N_MODES * NX * NY            # 36864
ROWS_PER_CORE = ROWS_TOTAL // N_CORES     # 4608
FUSE = 4                                  # systems per partition row
GROUP_ROWS = 128 * FUSE                   # 512 systems per group
N_GROUPS = ROWS_PER_CORE // GROUP_ROWS    # 9
FD = FUSE * NV                            # 2048
T0 = 16                                   # head length per system
HD = FUSE * T0

_V = (np.arange(NV, dtype=np.float64) + 1.0) * DV

# f32 const blob: resetv [FD], then reset1h, pw2kh, g1wh, g2wh [HD each]
CF_W = FD + 4 * HD


def _profiles():
    v = _V
    vh = v[:T0]
    g1 = 3.0 * v**2 - v**4 - 2.0 * v
    g2 = v**4 - v
    pwn = -KY / (2.0 * DV * v**3)         # wn' = -w/2  (0.5 folded in)
    pun = -KY / (DV * DV * v**2)          # un' = -u/2
    r1 = np.ones(NV)
    r1[1:] = v[:-1] / v[1:]
    r1[0] = 0.0                           # E1 reset at each system start
    r3 = np.ones(T0)
    r3[1:] = (vh[:-1] / vh[1:])**3
    r3[0] = 0.0
    r2 = np.ones(T0)
    r2[1:] = (vh[:-1] / vh[1:])**2
    r2[0] = 0.0
    return np.concatenate([
        np.tile(r1, FUSE),
        np.tile(r3, FUSE),
        np.tile(r2, FUSE),
        np.tile(0.5 * g1[:T0] * pwn[:T0], FUSE),
        np.tile(0.5 * g2[:T0] * pun[:T0], FUSE),
    ])


def _legalize_multiwait(nc):
    """Split instructions with >1 sync wait: keep one wait on the
    instruction, hoist the rest onto standalone InstEventSemaphore ops
    immediately before it on the same engine (this walrus accepts only one
    wait per instruction)."""
    n = [0]

    def fresh(engine, wait):
        n[0] += 1
        return mybir.InstEventSemaphore(
            name=f"mwsplit-{n[0]}",
            engine=engine,
            sync_info=mybir.SyncInfo(on_wait=[wait], on_update=[]),
        )

    for fn in nc.m.functions:
        for blk in fn.blocks:
            out = []
            for ins in blk.instructions:
                si = ins.sync_info
                if si is not None and si.on_wait is not None and len(si.on_wait) > 1:
                    waits = list(si.on_wait)
                    for w in waits[:-1]:
                        out.append(fresh(ins.engine, w))
                    si.on_wait = [waits[-1]]
                out.append(ins)
            blk.instructions[:] = out


def build_nc(n_groups=N_GROUPS, legalize=True):
    nc = bass.Bass()
    rows = n_groups * GROUP_ROWS
    y_in = nc.declare_dram_parameter("y", [rows, NV], F32, isOutput=False)
    cf_in = nc.declare_dram_parameter("cf", [128, CF_W], F32, isOutput=False)
    ilp_in = nc.declare_dram_parameter("ilp", [128, n_groups * HD], F32, isOutput=False)
    out_ext = nc.declare_dram_parameter("out", [rows, NV], F32, isOutput=True)

    MUL = mybir.AluOpType.mult
    ADD = mybir.AluOpType.add
    SUB = mybir.AluOpType.subtract
    COPY = mybir.ActivationFunctionType.Copy

    pw0 = float(-KY / (2.0 * DV * _V[0]**3))
    pu0 = float(-KY / (DV * DV * _V[0]**2))
    vlast = float(_V[-1])

    with ExitStack() as ctx:
        tc = ctx.enter_context(tile.TileContext(nc))
        cpool = ctx.enter_context(tc.tile_pool(name="consts", bufs=1))

        cf = cpool.tile([128, CF_W], F32, tag="cf")
        nc.gpsimd.dma_start(cf[:, :], cf_in[:, :])
        ilp = cpool.tile([128, n_groups * HD], F32, tag="ilp")
        nc.gpsimd.dma_start(ilp[:, :], ilp_in[:, :])

        resetv = cf[:, 0:FD]
        reset1h = cf[:, FD:FD + HD]
        pw2kh = cf[:, FD + HD:FD + 2 * HD]
        g1wh = cf[:, FD + 2 * HD:FD + 3 * HD]
        g2wh = cf[:, FD + 3 * HD:FD + 4 * HD]

        twos = cpool.tile([128, HD], F32, tag="twos")
        nc.gpsimd.memset(twos[:, :], 2.0)

        # touch consts so the tile framework orders compute after the loads
        for nm, seg in (("tc_f", cf), ("tc_i", ilp)):
            tch = cpool.tile([128, 1], F32, tag=nm)
            nc.vector.tensor_copy(out=tch[:, :], in_=seg[:, 0:1])

        io = ctx.enter_context(tc.tile_pool(name="io", bufs=6))
        e1p = ctx.enter_context(tc.tile_pool(name="e1", bufs=2))
        wk = ctx.enter_context(tc.tile_pool(name="work", bufs=3))

        # --- 3-stage software pipeline -------------------------------
        # A(g): input DMA, E1 scan, S1 seeds, head compaction, weighted
        #       scans wn/un, t3.
        # B(g): diagonal/off-diagonal assembly, reciprocal, alpha/mcp/beta.
        # C(g): dp/xb solve scans, scatter, output DMA.
        # Issuing A(g), C(g-2), B(g-1) keeps every in-order engine queue
        # stocked with ready work (the g-2 solve depends only on stage-B
        # results from the previous iteration).
        st = {}

        def stage_a(g):
            rsl = slice(g * GROUP_ROWS, (g + 1) * GROUP_ROWS)
            y_src = y_in[rsl, :].rearrange("(p j) v -> p (j v)", p=128)
            d = {}
            y4 = io.tile([128, FD], F32, tag="y4")
            (nc.scalar if g == 4 else nc.sync).dma_start(y4[:, :], y_src)
            d["y4"] = y4
            y4v = y4[:, :].rearrange("p (j v) -> p j v", j=FUSE)
            d["y4v"] = y4v

            # S1 per system via full-row ratio scan (E1_t = P(y*v)_t / v_t)
            E1 = e1p.tile([128, FD], F32, tag="E1")
            nc.vector.tensor_tensor_scan(E1[:, :], resetv, y4[:, :], 0.0,
                                         op0=MUL, op1=ADD)
            s1x2 = wk.tile([128, FUSE], F32, tag="s1x2")
            nc.scalar.activation(s1x2[:, :], E1[:, NV - 1::NV], COPY,
                                 bias=0.0, scale=pw0 * vlast)
            s1xp = wk.tile([128, FUSE], F32, tag="s1xp")
            nc.scalar.activation(s1xp[:, :], E1[:, NV - 1::NV], COPY,
                                 bias=0.0, scale=0.5 * pu0 * vlast)

            yh = wk.tile([128, HD], F32, tag="yh")
            yhv = yh[:, :].rearrange("p (j v) -> p j v", j=FUSE)
            nc.gpsimd.tensor_copy(out=yhv[:, :, :], in_=y4v[:, :, 0:T0])
            d["yh"] = yh

            wg1 = wk.tile([128, HD], F32, tag="wg1")
            nc.gpsimd.tensor_tensor(out=wg1[:, :], in0=yh[:, :], in1=g1wh, op=MUL)
            wg2 = wk.tile([128, HD], F32, tag="wg2")
            nc.gpsimd.tensor_tensor(out=wg2[:, :], in0=yh[:, :], in1=g2wh, op=MUL)
            nc.gpsimd.tensor_tensor(out=wg1[:, 0::T0], in0=wg1[:, 0::T0],
                                    in1=s1x2[:, :], op=ADD)
            nc.gpsimd.tensor_tensor(out=wg2[:, 0::T0], in0=wg2[:, 0::T0],
                                    in1=s1xp[:, :], op=ADD)

            wn = wk.tile([128, HD], F32, tag="wn")
            nc.vector.tensor_tensor_scan(wn[:, :], reset1h, wg1[:, :], 0.0,
                                         op0=MUL, op1=ADD)
            un = wk.tile([128, HD], F32, tag="un")
            nc.vector.tensor_tensor_scan(un[:, :], pw2kh, wg2[:, :], 0.0,
                                         op0=MUL, op1=ADD)
            d["wn"], d["un"] = wn, un

            t3 = wk.tile([128, HD], F32, tag="t3")
            nc.scalar.activation(t3[:, :], yh[:, :], COPY,
                                 bias=1.0, scale=float(8.0 * np.pi * Y_DT))
            d["t3"] = t3
            return d

        def stage_b(g, d):
            wn, un, yh, t3 = d["wn"], d["un"], d["yh"], d["t3"]
            b1 = wk.tile([128, HD], F32, tag="b1")
            nc.gpsimd.tensor_tensor(out=b1[:, :], in0=t3[:, :], in1=un[:, :], op=SUB)
            bil = wk.tile([128, HD], F32, tag="bil")
            nc.gpsimd.tensor_tensor(out=bil[:, :], in0=wn[:, :],
                                    in1=ilp[:, g * HD:(g + 1) * HD], op=MUL)
            bpos = wk.tile([128, HD], F32, tag="bpos")
            nc.gpsimd.tensor_tensor(out=bpos[:, :], in0=b1[:, :], in1=bil[:, :], op=ADD)
            binv = wk.tile([128, HD], F32, tag="binv")
            nc.vector.reciprocal(out=binv[:, :], in_=bpos[:, :])
            binv2 = wk.tile([128, HD], F32, tag="binv2")
            nc.gpsimd.tensor_tensor(out=binv2[:, :], in0=binv[:, :], in1=twos[:, :], op=MUL)

            a_n = wk.tile([128, HD], F32, tag="a_n")
            nc.gpsimd.tensor_tensor(out=a_n[:, :], in0=un[:, :], in1=wn[:, :], op=SUB)
            c_n = wk.tile([128, HD], F32, tag="c_n")
            nc.gpsimd.tensor_tensor(out=c_n[:, :], in0=un[:, :], in1=wn[:, :], op=ADD)

            alpha = wk.tile([128, HD], F32, tag="alpha")    # -a/b
            nc.gpsimd.tensor_tensor(out=alpha[:, :], in0=a_n[:, :], in1=binv2[:, :], op=MUL)
            av = alpha[:, :].rearrange("p (j v) -> p j v", j=FUSE)
            nc.gpsimd.memset(av[:, :, 0:1], 0.0)            # fwd scan reset
            mcp = wk.tile([128, HD], F32, tag="mcp")        # -c/b
            nc.gpsimd.tensor_tensor(out=mcp[:, :], in0=c_n[:, :], in1=binv2[:, :], op=MUL)
            mv = mcp[:, :].rearrange("p (j v) -> p j v", j=FUSE)
            nc.gpsimd.memset(mv[:, :, T0 - 1:T0], 0.0)      # bwd scan reset
            beta = wk.tile([128, HD], F32, tag="beta")      # y/b
            nc.gpsimd.tensor_tensor(out=beta[:, :], in0=yh[:, :], in1=binv[:, :], op=MUL)
            d["alpha"], d["mcp"], d["beta"] = alpha, mcp, beta

        def stage_c(g, d):
            rsl = slice(g * GROUP_ROWS, (g + 1) * GROUP_ROWS)
            x_dst = out_ext[rsl, :].rearrange("(p j) v -> p (j v)", p=128)
            alpha, mcp, beta = d["alpha"], d["mcp"], d["beta"]
            dp = wk.tile([128, HD], F32, tag="dp")
            nc.vector.tensor_tensor_scan(dp[:, :], alpha[:, :], beta[:, :], 0.0,
                                         op0=MUL, op1=ADD)
            xh = wk.tile([128, HD], F32, tag="xh")
            nc.vector.tensor_tensor_scan(xh[:, ::-1], mcp[:, ::-1], dp[:, ::-1], 0.0,
                                         op0=MUL, op1=ADD)
            xhv = xh[:, :].rearrange("p (j v) -> p j v", j=FUSE)
            nc.gpsimd.tensor_copy(out=d["y4v"][:, :, 0:T0], in_=xhv[:, :, :])
            if g >= n_groups - 2:
                h = FD // 2
                nc.sync.dma_start(x_dst[:, 0:h], d["y4"][:, 0:h])
                eng = nc.scalar if g % 2 == 0 else nc.gpsimd
                eng.dma_start(x_dst[:, h:FD], d["y4"][:, h:FD])
            elif g % 2 == 1:
                nc.gpsimd.dma_start(x_dst, d["y4"][:, :])
            else:
                nc.scalar.dma_start(x_dst, d["y4"][:, :])

        for g in range(n_groups + 2):
            if g < n_groups:
                st[g] = stage_a(g)
            if g - 2 >= 0:
                stage_c(g - 2, st[g - 2])
                del st[g - 2]
            if g - 1 < n_groups and g - 1 >= 0:
                stage_b(g - 1, st[g - 1])

    if legalize:
        _legalize_multiwait(nc)
    return nc


_NC_CACHE = {}


def _get_nc(n_groups=N_GROUPS):
    if n_groups not in _NC_CACHE:
        _NC_CACHE[n_groups] = build_nc(n_groups)
    return _NC_CACHE[n_groups]


_CF_CACHE = None


def make_inputs(y_shard, il2_rows, n_groups=N_GROUPS):
    """Per-core input map. y_shard [rows, 512] f32; il2_rows [rows] f32
    (holding il*(il+1)/2 per row)."""
    global _CF_CACHE
    if _CF_CACHE is None:
        _CF_CACHE = np.broadcast_to(_profiles()[None, :], (128, CF_W)
                                    ).astype(np.float32).copy()
    il2 = il2_rows.reshape(n_groups, 128, FUSE)[:, :, 0]          # [g, 128]
    prof = (4.0 * DV / _V[:T0]).astype(np.float64)                # [T0]
    ilp = (il2[:, :, None, None] * prof[None, None, None, :])     # [g,128,1,T0]
    ilp = np.broadcast_to(ilp, (n_groups, 128, FUSE, T0))
    ilp = ilp.transpose(1, 0, 2, 3).reshape(128, n_groups * HD).astype(np.float32)
    return {
        "y": np.ascontiguousarray(y_shard, dtype=np.float32),
        "cf": _CF_CACHE,
        "ilp": np.ascontiguousarray(ilp),
    }


def kernel(y, il_arr):
    y = np.asarray(y, dtype=np.float32)
    il_arr = np.asarray(il_arr)
    yf = y.reshape(ROWS_TOTAL, NV)
    il_f = il_arr.astype(np.float64)
    il2_all = np.repeat(il_f * (il_f + 1.0) / 2.0, NX * NY).astype(np.float32)

    nc = _get_nc()
    in_maps = []
    for c in range(N_CORES):
        rs = slice(c * ROWS_PER_CORE, (c + 1) * ROWS_PER_CORE)
        in_maps.append(make_inputs(yf[rs], il2_all[rs]))
    res = run_bass_kernel_spmd(nc, in_maps, core_ids=list(range(N_CORES)))
    outs = [res.results[c]["out"] for c in range(N_CORES)]
    x = np.concatenate(outs, axis=0).reshape(N_MODES, NX, NY, NV)
    return x.astype(np.float32)
